# revision 15
# baseline (speedup 1.0000x reference)
"""MoE gating kernel for Trainium2 (Bass/Tile), data-parallel over 8 NeuronCores.

Computes: logits = x @ W_g.T ; top-2 values; softmax over the 2 values.
  p1 = sigmoid(v1 - v2), p2 = sigmoid(v2 - v1)  (v1 >= v2 the top-2 logits)

Sharding: tokens split 8 ways (2048 tokens/core), W_g replicated.

v8 design notes:
  - x streams in per 256-token pair: pair 0 as fp32 via HWDGE/sync (first
    byte ~2.6us -- SWDGE's Q7 path needs ~5us to boot), pairs 1-7 via
    SWDGE/gpsimd with fp32->bf16 cast during DMA. HBM reads (the ~45us
    roofline) are unchanged either way.
  - W_g arrives pre-transposed/cast to bf16 from the host (weight layout
    prep), along with bf16/fp32 identities and a zero tile, all via sync.
  - transposes are REGULAR matmuls against an identity moving operand
    (out = x_block.T @ I), NOT transpose-mode: the HAM clock gate ignores
    transpose-mode passes and re-throttles the PE to 1.2GHz across long
    transpose phases (~20-40us at half clock in earlier versions).
    Regular bf16 transposes sustain ~56ns warm (FWL + background-buffer
    weight loads). Their PSUM output is fp32 (TRN2 rule), so drains are
    split 3:5 between DVE and the otherwise-idle ACT engine, casting to
    bf16 on the way out.
  - N=512 warm-up matmuls on a zero tile flip HAM at ~+6us (N<=256 never
    flips it); small keeper blocks at late group boundaries split the
    PE's data-wait idle below HAM's warm MID window (1.7us) so the clock
    never drops mid-kernel.
  - sigmoids batch into one ACT call at the end (no Copy<->Sigmoid table
    switches mid-kernel) and the output leaves partition-major [128, 32]
    in ONE contiguous DMA (128 descriptors -- a token-major store is 2048
    8-byte descriptors whose HBM write receipts add ~6us before the final
    semaphore); the host de-interleaves.
bf16 adds ~4e-3 relative error on the output probabilities (gate is 2e-2).
"""

import sys

sys.path.insert(0, "/opt/trn_rl_repo")

from contextlib import ExitStack

import numpy as np
import ml_dtypes

import concourse.bass as bass
import concourse.bacc as bacc
import concourse.mybir as mybir
from concourse.tile import TileContext
from concourse.bass_utils import run_bass_kernel_spmd

TOKENS = 16384
DIM = 2048
E = 64  # num experts
NCORES = 8
TPC = TOKENS // NCORES  # tokens per core
P = 128
KT = DIM // P  # 16 contraction tiles
G = 256  # token group (moving-dim of the big matmul)
NG = TPC // G  # 8 groups per core
TB = G // P  # 2 token blocks per group
NB = NG * TB  # 16 token blocks per core

F32 = mybir.dt.float32
BF16 = mybir.dt.bfloat16
N_WARM = 12  # warm-up matmuls; N=512 gives the ~80% PE duty HAM needs to flip


def _emit(tc, ctx, x_ap, wgt_ap, idb_ap, idf_ap, wz_ap, out_ap):
    nc = tc.nc

    singles = ctx.enter_context(tc.tile_pool(name="singles", bufs=1))
    xpool = ctx.enter_context(tc.tile_pool(name="xpool", bufs=1))
    xtpool = ctx.enter_context(tc.tile_pool(name="xtpool", bufs=3))
    ltpool = ctx.enter_context(tc.tile_pool(name="ltpool", bufs=2))
    spool = ctx.enter_context(tc.tile_pool(name="spool", bufs=4))
    psum_t = ctx.enter_context(tc.tile_pool(name="psum_t", bufs=4, space="PSUM"))
    psum_l = ctx.enter_context(tc.tile_pool(name="psum_l", bufs=2, space="PSUM"))
    psum_f = ctx.enter_context(tc.tile_pool(name="psum_f", bufs=1, space="PSUM"))
    psum_w = ctx.enter_context(tc.tile_pool(name="psum_w", bufs=1, space="PSUM"))

    # sync (HWDGE) ring, in order: zero tile (warm-up operands, ~+3us),
    # pair 0 of x as fp32 (stream starts ~+2.6us), identities, wgT.
    wz = singles.tile([P, 5 * P], BF16)
    nc.sync.dma_start(out=wz[:], in_=wz_ap)
    px0 = xpool.tile([P, TB, DIM], F32, tag="x0")
    nc.sync.dma_start(
        out=px0[:], in_=x_ap[0:G, :].rearrange("(s p) d -> p s d", p=P)
    )
    ident = singles.tile([P, P], BF16)
    nc.sync.dma_start(out=ident[:], in_=idb_ap)
    ident_f = singles.tile([P, P], F32)
    nc.sync.dma_start(out=ident_f[:], in_=idf_ap)
    wgT = singles.tile([P, KT, E], BF16)
    nc.sync.dma_start(out=wgT[:], in_=wgt_ap)

    # x pairs 1..7 via SWDGE (gpsimd), bf16 cast during DMA
    all_x = [px0]
    for g in range(1, NG):
        px = xpool.tile([P, TB, DIM], BF16, tag=f"x{g}")
        all_x.append(px)
        nc.gpsimd.dma_start(
            out=px[:],
            in_=x_ap[g * G : (g + 1) * G, :].rearrange("(s p) d -> p s d", p=P),
        )

    # PE warm-up from boot until first x data lands: flips the HAM clock
    # gate to 2.4GHz before real work starts.
    warm = wz[:, :P]
    warm_rhs = wz[:, P : 5 * P]
    for _ in range(N_WARM):
        pw = psum_w.tile([P, 4 * P], F32, tag="warm_ps")
        nc.tensor.matmul(pw[:], warm, warm_rhs)

    def keeper(n=1):
        # countable PE work with no dependencies: splits the PE's
        # data-wait idle below HAM's warm MID window; ~266ns each
        for _ in range(n):
            pw = psum_w.tile([P, 4 * P], F32, tag="warm_ps")
            nc.tensor.matmul(pw[:], warm, warm_rhs)

    # per-token-block v1-v2 / v2-v1 accumulate here; one sigmoid + one
    # contiguous partition-major store at the end
    dd_all = singles.tile([P, NB, 2], F32)

    def epilogue(g, lp):
        # back to token-major + top-2 (runs one group late)
        lt = ltpool.tile([E, G], F32)
        for tb in range(TB):
            nc.vector.tensor_copy(
                lt[:, tb * P : (tb + 1) * P], lp[:, tb * P : (tb + 1) * P]
            )
            fp = psum_f.tile([P, E], F32, tag="fin_ps")
            nc.tensor.matmul(
                fp[:],
                lt[:, tb * P : (tb + 1) * P],
                ident_f[:E, :E],
                is_transpose=True,
            )
            max8 = spool.tile([P, 8], F32)
            nc.vector.max(out=max8[:], in_=fp[:])
            b = g * TB + tb
            nc.vector.tensor_sub(dd_all[:, b, 0:1], max8[:, 0:1], max8[:, 1:2])
            nc.vector.tensor_sub(dd_all[:, b, 1:2], max8[:, 1:2], max8[:, 0:1])

    pending = None  # (g, lp) awaiting epilogue
    for g in range(NG):
        px = all_x[g]
        idg = ident_f if g == 0 else ident

        if g >= 3:
            keeper(2)

        # transpose into xT [128 d, k * G t] (bf16 in SBUF, fp32 in PSUM).
        # Each 2KB PSUM bank holds 2 k-slices x 2 token blocks; drains
        # split 3:5 between DVE and ACT, casting fp32 -> bf16.
        xt = xtpool.tile([P, KT * G], BF16)
        for q in range(KT // 2):
            pt = psum_t.tile([P, 2 * G], F32)
            for dk in range(2):
                k = 2 * q + dk
                for tb in range(TB):
                    nc.tensor.matmul(
                        pt[:, dk * G + tb * P : dk * G + (tb + 1) * P],
                        px[:, tb, k * P : (k + 1) * P],
                        idg[:],
                    )
            dst = xt[:, 2 * q * G : (2 * q + 2) * G]
            if q < 3:
                nc.vector.tensor_copy(dst, pt[:])
            else:
                nc.scalar.copy(dst, pt[:])

        # logitsT [64 e, 256 t] = sum_k wgT_k.T @ xT_k  (bf16 -> fp32 PSUM)
        lp = psum_l.tile([E, G], F32)
        for k in range(KT):
            nc.tensor.matmul(
                lp[:],
                wgT[:, k, :],
                xt[:, k * G : (k + 1) * G],
                start=(k == 0),
                stop=(k == KT - 1),
            )

        if pending is not None:
            epilogue(*pending)
        pending = (g, lp)
    epilogue(*pending)

    # single sigmoid + one contiguous partition-major store
    ot = singles.tile([P, NB, 2], F32)
    nc.scalar.activation(ot[:], dd_all[:], mybir.ActivationFunctionType.Sigmoid)
    nc.sync.dma_start(out=out_ap, in_=ot[:])


_NC_CACHE = {}


def _build():
    key = "nc"
    if key in _NC_CACHE:
        return _NC_CACHE[key]
    nc = bacc.Bacc(trn_type="TRN2")
    x = nc.dram_tensor("x", [TPC, DIM], F32, kind="ExternalInput")
    wgt = nc.dram_tensor("wgt", [P, KT * E], BF16, kind="ExternalInput")
    idb = nc.dram_tensor("idb", [P, P], BF16, kind="ExternalInput")
    idf = nc.dram_tensor("idf", [P, P], F32, kind="ExternalInput")
    wz = nc.dram_tensor("wz", [P, 5 * P], BF16, kind="ExternalInput")
    out = nc.dram_tensor("out", [P, NB * 2], F32, kind="ExternalOutput")
    with TileContext(nc) as tc, ExitStack() as ctx:
        _emit(tc, ctx, x.ap(), wgt.ap(), idb.ap(), idf.ap(), wz.ap(), out.ap())
    if not nc.is_finalized():
        nc.finalize()
    _NC_CACHE[key] = nc
    return nc


def _run(x, W_g, trace=False):
    nc = _build()
    x = np.ascontiguousarray(np.asarray(x, dtype=np.float32))
    W_g = np.asarray(W_g, dtype=np.float32)
    # host-side weight layout prep: wgt[p, k*E + e] = W_g[e, k*128 + p]
    wgt = np.ascontiguousarray(
        W_g.reshape(E, KT, P).transpose(2, 1, 0).reshape(P, KT * E)
    ).astype(ml_dtypes.bfloat16)
    idb = np.eye(P, dtype=np.float32).astype(ml_dtypes.bfloat16)
    idf = np.eye(P, dtype=np.float32)
    wz = np.zeros((P, 5 * P), dtype=ml_dtypes.bfloat16)
    in_maps = [
        {
            "x": np.ascontiguousarray(x[c * TPC : (c + 1) * TPC]),
            "wgt": wgt,
            "idb": idb,
            "idf": idf,
            "wz": wz,
        }
        for c in range(NCORES)
    ]
    res = run_bass_kernel_spmd(nc, in_maps, core_ids=list(range(NCORES)), trace=trace)
    # device output is partition-major [128, 16, 2]; de-interleave:
    # out[b*128 + p, :] = res[p, b, :]
    outs = []
    for r in res.results:
        o = r["out"].reshape(P, NB, 2).transpose(1, 0, 2).reshape(TPC, 2)
        outs.append(o)
    out = np.ascontiguousarray(np.concatenate(outs, axis=0))
    return out, res


def kernel(x, W_g):
    out, _ = _run(x, W_g, trace=False)
    return out


def kernel_profiled(x, W_g, **_kw):
    out, res = _run(x, W_g, trace=True)
    return out, res


# revision 17
# speedup vs baseline: 1.2678x; 1.2678x over previous
"""MoE gating kernel for Trainium2 (Bass/Tile), data-parallel over 8 NeuronCores.

Computes: logits = x @ W_g.T ; top-2 values; softmax over the 2 values.
  p1 = sigmoid(v1 - v2), p2 = sigmoid(v2 - v1)  (v1 >= v2 the top-2 logits)

Sharding: tokens split 8 ways (2048 tokens/core), W_g replicated.

v8 design notes:
  - x streams in per 256-token pair: pair 0 as fp32 via HWDGE/sync (first
    byte ~2.6us -- SWDGE's Q7 path needs ~5us to boot), pairs 1-7 via
    SWDGE/gpsimd with fp32->bf16 cast during DMA. HBM reads (the ~45us
    roofline) are unchanged either way.
  - W_g arrives pre-transposed/cast to bf16 from the host (weight layout
    prep), along with bf16/fp32 identities and a zero tile, all via sync.
  - transposes are REGULAR matmuls against an identity moving operand
    (out = x_block.T @ I), NOT transpose-mode: the HAM clock gate ignores
    transpose-mode passes and re-throttles the PE to 1.2GHz across long
    transpose phases (~20-40us at half clock in earlier versions).
    Regular bf16 transposes sustain ~56ns warm (FWL + background-buffer
    weight loads). Their PSUM output is fp32 (TRN2 rule), so drains are
    split 3:5 between DVE and the otherwise-idle ACT engine, casting to
    bf16 on the way out.
  - N=512 warm-up matmuls on a zero tile flip HAM at ~+6us (N<=256 never
    flips it); small keeper blocks at late group boundaries split the
    PE's data-wait idle below HAM's warm MID window (1.7us) so the clock
    never drops mid-kernel.
  - sigmoids batch into one ACT call at the end (no Copy<->Sigmoid table
    switches mid-kernel) and the output leaves partition-major [128, 32]
    in ONE contiguous DMA (128 descriptors -- a token-major store is 2048
    8-byte descriptors whose HBM write receipts add ~6us before the final
    semaphore); the host de-interleaves.
bf16 adds ~4e-3 relative error on the output probabilities (gate is 2e-2).
"""

import sys

sys.path.insert(0, "/opt/trn_rl_repo")

from contextlib import ExitStack

import numpy as np
import ml_dtypes

import concourse.bass as bass
import concourse.bacc as bacc
import concourse.mybir as mybir
from concourse.tile import TileContext
from concourse.bass_utils import run_bass_kernel_spmd

TOKENS = 16384
DIM = 2048
E = 64  # num experts
NCORES = 8
TPC = TOKENS // NCORES  # tokens per core
P = 128
KT = DIM // P  # 16 contraction tiles
G = 256  # token group (moving-dim of the big matmul)
NG = TPC // G  # 8 groups per core
TB = G // P  # 2 token blocks per group
NB = NG * TB  # 16 token blocks per core

F32 = mybir.dt.float32
BF16 = mybir.dt.bfloat16
N_WARM = 12  # warm-up matmuls; N=512 gives the ~80% PE duty HAM needs to flip


def _emit(tc, ctx, x_ap, wgt_ap, idb_ap, idf_ap, wz_ap, out_ap):
    nc = tc.nc

    singles = ctx.enter_context(tc.tile_pool(name="singles", bufs=1))
    xpool = ctx.enter_context(tc.tile_pool(name="xpool", bufs=1))
    xtpool = ctx.enter_context(tc.tile_pool(name="xtpool", bufs=3))
    ltpool = ctx.enter_context(tc.tile_pool(name="ltpool", bufs=2))
    spool = ctx.enter_context(tc.tile_pool(name="spool", bufs=4))
    psum_t = ctx.enter_context(tc.tile_pool(name="psum_t", bufs=4, space="PSUM"))
    psum_l = ctx.enter_context(tc.tile_pool(name="psum_l", bufs=2, space="PSUM"))
    psum_f = ctx.enter_context(tc.tile_pool(name="psum_f", bufs=1, space="PSUM"))
    psum_w = ctx.enter_context(tc.tile_pool(name="psum_w", bufs=1, space="PSUM"))

    # sync (HWDGE) ring, in order: zero tile (warm-up operands, ~+3us),
    # identities, wgT -- all constants land by ~+4us.
    wz = singles.tile([P, 5 * P], BF16)
    nc.sync.dma_start(out=wz[:], in_=wz_ap)
    ident = singles.tile([P, P], BF16)
    nc.sync.dma_start(out=ident[:], in_=idb_ap)
    ident_f = singles.tile([P, P], F32)
    nc.sync.dma_start(out=ident_f[:], in_=idf_ap)
    wgT = singles.tile([P, KT, E], BF16)
    nc.sync.dma_start(out=wgT[:], in_=wgt_ap)

    # x pairs via SWDGE (gpsimd), bf16 cast during DMA
    all_x = []
    for g in range(NG):
        px = xpool.tile([P, TB, DIM], BF16, tag=f"x{g}")
        all_x.append(px)
        nc.gpsimd.dma_start(
            out=px[:],
            in_=x_ap[g * G : (g + 1) * G, :].rearrange("(s p) d -> p s d", p=P),
        )

    # PE warm-up from boot until first x data lands: flips the HAM clock
    # gate to 2.4GHz before real work starts.
    warm = wz[:, :P]
    warm_rhs = wz[:, P : 5 * P]
    for _ in range(N_WARM):
        pw = psum_w.tile([P, 4 * P], F32, tag="warm_ps")
        nc.tensor.matmul(pw[:], warm, warm_rhs)

    def keeper(n=1):
        # countable PE work with no dependencies: splits the PE's
        # data-wait idle below HAM's warm MID window; ~266ns each
        for _ in range(n):
            pw = psum_w.tile([P, 4 * P], F32, tag="warm_ps")
            nc.tensor.matmul(pw[:], warm, warm_rhs)

    # per-token-block v1-v2 / v2-v1 accumulate here; one sigmoid + one
    # contiguous partition-major store at the end
    dd_all = singles.tile([P, NB, 2], F32)

    def epilogue(g, lp):
        # back to token-major + top-2 (runs one group late)
        lt = ltpool.tile([E, G], F32)
        for tb in range(TB):
            nc.vector.tensor_copy(
                lt[:, tb * P : (tb + 1) * P], lp[:, tb * P : (tb + 1) * P]
            )
            fp = psum_f.tile([P, E], F32, tag="fin_ps")
            nc.tensor.matmul(
                fp[:],
                lt[:, tb * P : (tb + 1) * P],
                ident_f[:E, :E],
                is_transpose=True,
            )
            max8 = spool.tile([P, 8], F32)
            nc.vector.max(out=max8[:], in_=fp[:])
            b = g * TB + tb
            nc.vector.tensor_sub(dd_all[:, b, 0:1], max8[:, 0:1], max8[:, 1:2])
            nc.vector.tensor_sub(dd_all[:, b, 1:2], max8[:, 1:2], max8[:, 0:1])

    pending = None  # (g, lp) awaiting epilogue
    for g in range(NG):
        px = all_x[g]
        idg = ident

        if g >= 1:
            keeper(3)

        # transpose into xT [128 d, k * G t] (bf16 in SBUF, fp32 in PSUM).
        # Each 2KB PSUM bank holds 2 k-slices x 2 token blocks; drains
        # split 3:5 between DVE and ACT, casting fp32 -> bf16.
        xt = xtpool.tile([P, KT * G], BF16)
        for q in range(KT // 2):
            pt = psum_t.tile([P, 2 * G], F32)
            for dk in range(2):
                k = 2 * q + dk
                for tb in range(TB):
                    nc.tensor.matmul(
                        pt[:, dk * G + tb * P : dk * G + (tb + 1) * P],
                        px[:, tb, k * P : (k + 1) * P],
                        idg[:],
                    )
            dst = xt[:, 2 * q * G : (2 * q + 2) * G]
            if q < 3:
                nc.vector.tensor_copy(dst, pt[:])
            else:
                nc.scalar.copy(dst, pt[:])

        # logitsT [64 e, 256 t] = sum_k wgT_k.T @ xT_k  (bf16 -> fp32 PSUM)
        lp = psum_l.tile([E, G], F32)
        for k in range(KT):
            nc.tensor.matmul(
                lp[:],
                wgT[:, k, :],
                xt[:, k * G : (k + 1) * G],
                start=(k == 0),
                stop=(k == KT - 1),
            )

        if pending is not None:
            epilogue(*pending)
        pending = (g, lp)
    epilogue(*pending)

    # single sigmoid + one contiguous partition-major store
    ot = singles.tile([P, NB, 2], F32)
    nc.scalar.activation(ot[:], dd_all[:], mybir.ActivationFunctionType.Sigmoid)
    nc.sync.dma_start(out=out_ap, in_=ot[:])


_NC_CACHE = {}


def _build():
    key = "nc"
    if key in _NC_CACHE:
        return _NC_CACHE[key]
    nc = bacc.Bacc(trn_type="TRN2")
    x = nc.dram_tensor("x", [TPC, DIM], F32, kind="ExternalInput")
    wgt = nc.dram_tensor("wgt", [P, KT * E], BF16, kind="ExternalInput")
    idb = nc.dram_tensor("idb", [P, P], BF16, kind="ExternalInput")
    idf = nc.dram_tensor("idf", [P, P], F32, kind="ExternalInput")
    wz = nc.dram_tensor("wz", [P, 5 * P], BF16, kind="ExternalInput")
    out = nc.dram_tensor("out", [P, NB * 2], F32, kind="ExternalOutput")
    with TileContext(nc) as tc, ExitStack() as ctx:
        _emit(tc, ctx, x.ap(), wgt.ap(), idb.ap(), idf.ap(), wz.ap(), out.ap())
    if not nc.is_finalized():
        nc.finalize()
    _NC_CACHE[key] = nc
    return nc


def _run(x, W_g, trace=False):
    nc = _build()
    x = np.ascontiguousarray(np.asarray(x, dtype=np.float32))
    W_g = np.asarray(W_g, dtype=np.float32)
    # host-side weight layout prep: wgt[p, k*E + e] = W_g[e, k*128 + p]
    wgt = np.ascontiguousarray(
        W_g.reshape(E, KT, P).transpose(2, 1, 0).reshape(P, KT * E)
    ).astype(ml_dtypes.bfloat16)
    idb = np.eye(P, dtype=np.float32).astype(ml_dtypes.bfloat16)
    idf = np.eye(P, dtype=np.float32)
    wz = np.zeros((P, 5 * P), dtype=ml_dtypes.bfloat16)
    in_maps = [
        {
            "x": np.ascontiguousarray(x[c * TPC : (c + 1) * TPC]),
            "wgt": wgt,
            "idb": idb,
            "idf": idf,
            "wz": wz,
        }
        for c in range(NCORES)
    ]
    res = run_bass_kernel_spmd(nc, in_maps, core_ids=list(range(NCORES)), trace=trace)
    # device output is partition-major [128, 16, 2]; de-interleave:
    # out[b*128 + p, :] = res[p, b, :]
    outs = []
    for r in res.results:
        o = r["out"].reshape(P, NB, 2).transpose(1, 0, 2).reshape(TPC, 2)
        outs.append(o)
    out = np.ascontiguousarray(np.concatenate(outs, axis=0))
    return out, res


def kernel(x, W_g):
    out, _ = _run(x, W_g, trace=False)
    return out


def kernel_profiled(x, W_g, **_kw):
    out, res = _run(x, W_g, trace=True)
    return out, res
